# revision 29
# baseline (speedup 1.0000x reference)
"""Dual-score causal attention on 8 Trainium2 NeuronCores.

Math (per batch*head):
    S = (q @ k.T + pe_q @ pe_k.T) * D**-0.5   == concat(q,pe_q) @ concat(k,pe_k).T * scale
    O = softmax(causal_mask(S)) @ v

Sharding: B*H = 32 pairs -> 4 per core (head/data parallel, no collectives).

Layout strategy (host-side shard step; pure data movement, no math):
  - Q' = [q|pe_q], K' = [k|pe_k] pre-transposed to d-major [128, L] f16 so the
    device streams them into SBUF at line rate (no on-device staging/transpose).
  - V laid out [128, NKB, 65] f16 with a ones-column baked in (row sums of
    exp(S) fall out of the A@V matmul chain -> softmax denominator).

Per-core kernel (engine-balanced, latency-hidden):
  - PE: S^T tiles [128 k x 512 q] fp16 -> PSUM; A^T@V accumulate [65, 512];
    O^T transposed back via identity matmul.
  - exp alternates between the scalar engine (table Exp) and VectorE
    (single-pass Schraudolph: ex_bits = i16(s*A + B) reinterpreted as f16)
    per stage, so both engines run concurrently at ~50% duty.  The softmax
    normalization cancels the approximation's bias (end-to-end ~7e-3 vs the
    2e-2 gate).
  - 3-deep PSUM score pipeline (stp bufs=3, LAG=3) hides the S->exp->AV
    cross-engine semaphore latency.  Diagonal (small) stages lead each
    q-block so the following full-size stages provide maximum latency cover.
  - Causal masking costs no elementwise work at all: one extra 128-col matmul
    per diagonal block accumulates -320 onto the upper triangle in PSUM
    (lhsT=diag(-320), rhs=upper-tri indicator).  exp then emits exact zeros:
    the scalar engine's Exp underflows, and the DVE path's f32->i16 convert
    saturates to -32768 whose f16 bitcast is -0.0 (verified on hardware).
  - A short burst of discarded matmuls during the input-DMA wait ramps the
    PE out of its cold p-state before the first real score matmul.
  - PSUM budget trick: the O^T transpose scratch aliases into the otp tile
    itself (already copied to SBUF by then): stp 3x2 + otp 2x1 = all 8 banks.
  - Drains are deferred one stage into the next q-block; outputs stream per
    q-block; next-bh input DMAs prefetch a full q-block ahead.
"""

import os
import sys

import numpy as np

B, H, L, D = 2, 16, 2048, 64
NCORES = 8
BHPC = (B * H) // NCORES  # bh pairs per core = 4
QB = 512  # query block (S^T free dim)
KB = 128  # key block (S^T partition dim)
NQB = L // QB  # 4
NKB = L // KB  # 16
KB_PER_QB = QB // KB  # 4
SCALE = float(D) ** -0.5
LOG2E = 1.4426950408889634
# Schraudolph-in-f16-bits: exp(s*SCALE) ~ bitcast_f16(int16(s*A + B))
SCHR_A = SCALE * LOG2E * 1024.0
SCHR_B = 15.0 * 1024.0

_CACHE = {}


def _import_concourse():
    try:
        import concourse  # noqa: F401
    except ImportError:
        for p in ("/opt/trn_rl_repo", "/root/.axon_site/_ro/trn_rl_repo"):
            if os.path.isdir(p) and p not in sys.path:
                sys.path.insert(0, p)


def _build_nc():
    """Build the single-core Bass program (same NEFF for all 8 cores)."""
    _import_concourse()
    from contextlib import ExitStack

    import concourse.tile as tile
    from concourse import bacc, mybir

    f32 = mybir.dt.float32
    f16 = mybir.dt.float16
    i16 = mybir.dt.int16

    nc = bacc.Bacc("TRN2", target_bir_lowering=False, debug=False)

    qT_d = nc.dram_tensor("qT", [BHPC, 128, L], f16, kind="ExternalInput").ap()
    kT_d = nc.dram_tensor("kT", [BHPC, 128, L], f16, kind="ExternalInput").ap()
    v_d = nc.dram_tensor("v", [BHPC, 128, NKB, D + 1], f16, kind="ExternalInput").ap()
    triu_d = nc.dram_tensor("triu", [128, 128], f16, kind="ExternalInput").ap()
    diagneg_d = nc.dram_tensor("diagneg", [128, 128], f16, kind="ExternalInput").ap()
    ident_d = nc.dram_tensor("ident", [128, 128], f32, kind="ExternalInput").ap()
    out_d = nc.dram_tensor("out", [BHPC, L, D], f32, kind="ExternalOutput").ap()

    Exp = mybir.ActivationFunctionType.Exp
    mult = mybir.AluOpType.mult
    add = mybir.AluOpType.add

    with tile.TileContext(nc) as tc:
        with ExitStack() as ctx:
            ep = ctx.enter_context

            const_pool = ep(tc.tile_pool(name="const", bufs=1))
            qT_pool = ep(tc.tile_pool(name="qT", bufs=2))
            kT_pool = ep(tc.tile_pool(name="kT", bufs=2))
            v_pool = ep(tc.tile_pool(name="v", bufs=2))
            ex_pool = ep(tc.tile_pool(name="ex", bufs=6))
            otsb_pool = ep(tc.tile_pool(name="otsb", bufs=2))
            ost_pool = ep(tc.tile_pool(name="ost", bufs=2))
            rc_pool = ep(tc.tile_pool(name="rc", bufs=4))
            stp_pool = ep(tc.tile_pool(name="stp", bufs=3, space="PSUM"))
            otp_pool = ep(tc.tile_pool(name="otp", bufs=2, space="PSUM"))

            triu = const_pool.tile([128, 128], f16)
            diagneg = const_pool.tile([128, 128], f16)
            ident = const_pool.tile([128, 128], f32)
            warm = const_pool.tile([128, 1], f32)
            nc.vector.memset(warm[:], 0.0)
            wpe = const_pool.tile([128, QB], f16)
            nc.vector.memset(wpe[:], 1.0)

            def emit_inputs(bh):
                qT = qT_pool.tile([128, L], f16)
                kT = kT_pool.tile([128, L], f16)
                vsb = v_pool.tile([128, NKB, D + 1], f16)
                if bh == 0:
                    # startup: what qi=0 stage-0 needs lands first, spread
                    # over two trigger queues so the transfers parallelize;
                    # the big qT chunk rides sync (earliest-clearing queue)
                    # scalar queue gets only the three tiny first-need
                    # transfers (so its exp table-load + first Exp aren't
                    # starved); everything else streams on sync
                    nc.sync.dma_start(qT[:, 0 : QB // 2], qT_d[bh, :, 0 : QB // 2])
                    nc.scalar.dma_start(kT[:, 0:KB], kT_d[bh, :, 0:KB])
                    nc.scalar.dma_start(triu[:], triu_d)
                    nc.scalar.dma_start(diagneg[:], diagneg_d)
                    nc.sync.dma_start(qT[:, QB // 2 : QB], qT_d[bh, :, QB // 2 : QB])
                    nc.scalar.dma_start(kT[:, KB:QB], kT_d[bh, :, KB:QB])
                    nc.scalar.dma_start(vsb[:], v_d[bh])
                    nc.scalar.dma_start(ident[:], ident_d)
                    nc.sync.dma_start(kT[:, QB:L], kT_d[bh, :, QB:L])
                    nc.sync.dma_start(qT[:, QB:L], qT_d[bh, :, QB:L])
                else:
                    nc.sync.dma_start(kT[:], kT_d[bh])
                    nc.sync.dma_start(qT[:], qT_d[bh])
                    nc.sync.dma_start(vsb[:], v_d[bh])
                return qT, kT, vsb

            pending_drain = [None]

            def flush_drain():
                if pending_drain[0] is not None:
                    pending_drain[0]()
                    pending_drain[0] = None

            stage_ctr = [0]  # global ACT/DVE alternation parity
            cur = emit_inputs(0)
            # preload the Exp activation table while the first DMAs land (the
            # implicit load otherwise delays the first real exp by ~1.4us);
            # emitted after the input triggers so it doesn't delay them
            nc.scalar.activation(warm[:], warm[:], Exp)
            # ramp the PE p-state during the input-DMA wait with discarded
            # self-contained matmuls so the first real score matmuls run at
            # full clock
            for _ in range(2):
                wstp = stp_pool.tile([128, 2 * QB], f32, tag="stp")
                for h_ in range(4):
                    nc.tensor.matmul(
                        wstp[:, (h_ % 2) * QB : (h_ % 2 + 1) * QB],
                        lhsT=wpe[:, 0:KB],
                        rhs=wpe[:],
                        start=True,
                        stop=True,
                        skip_group_check=True,
                    )
            nxt = [None]

            for bh in range(BHPC):
                qT, kT, vsb = cur
                ost = ost_pool.tile([128, NKB, D], f32)
                for qi in range(NQB):
                    # otp spans all 128 partitions: [0:65] holds the O^T
                    # accumulation; after its drain-copy the same bank is
                    # reused as transpose scratch (PSUM is only 8 banks)
                    otp = otp_pool.tile([128, QB], f32)
                    nfull = KB_PER_QB * qi  # fully-unmasked k-blocks

                    def emit_exp(ex, stp, ncols, is_dpair):
                        # -320 mask rows rely on the DVE f32->i16 convert
                        # SATURATING to -32768 (bitcast f16 -0.0)
                        on_dve = stage_ctr[0] % 2 == 1
                        stage_ctr[0] += 1
                        if on_dve:
                            nc.vector.tensor_scalar(
                                ex[:, 0:ncols].bitcast(i16),
                                stp[:, 0:ncols],
                                SCHR_A,
                                SCHR_B,
                                mult,
                                add,
                            )
                        else:
                            nc.scalar.activation(
                                ex[:, 0:ncols], stp[:, 0:ncols], Exp, scale=SCALE
                            )

                    # diagonal (small) stages lead: the following full-size
                    # stages then cover their exp/triangle latency, and the
                    # first dpair's r=0 matmul initializes the whole otp range
                    stages = [("dpair", r0) for r0 in range(0, KB_PER_QB, 2)]
                    stages += [("pair", j0) for j0 in range(0, nfull, 2)]

                    def emit_s(stage):
                        kind, a = stage
                        stp = stp_pool.tile([128, 2 * QB], f32, tag="stp")
                        ex = ex_pool.tile([128, 2 * QB], f16, tag="ex")
                        if kind == "pair":
                            for h_ in (0, 1):
                                j = a + h_
                                nc.tensor.matmul(
                                    stp[:, h_ * QB : (h_ + 1) * QB],
                                    lhsT=kT[:, j * KB : (j + 1) * KB],
                                    rhs=qT[:, qi * QB : (qi + 1) * QB],
                                    start=True,
                                    stop=True,
                                    skip_group_check=True,
                                )
                            emit_exp(ex, stp, 2 * QB, False)
                        else:
                            # two diagonal blocks r0, r0+1 packed into one
                            # exp: [0:na) for r0, [na:na+nb) for r0+1
                            off = 0
                            for r_ in (a, a + 1):
                                j = nfull + r_
                                m = KB * r_
                                n = QB - m
                                nc.tensor.matmul(
                                    stp[:, off : off + n],
                                    lhsT=kT[:, j * KB : (j + 1) * KB],
                                    rhs=qT[:, qi * QB + m : (qi + 1) * QB],
                                    start=True,
                                    stop=False,
                                    skip_group_check=True,
                                )
                                # causal mask: accumulate -320 onto the diag
                                # block's upper triangle (exp -> exact 0),
                                # replacing a VectorE/GpSimd multiply chain
                                nc.tensor.matmul(
                                    stp[:, off : off + KB],
                                    lhsT=diagneg[:],
                                    rhs=triu[:],
                                    start=False,
                                    stop=True,
                                    skip_group_check=True,
                                )
                                off += n
                            emit_exp(ex, stp, off, True)
                        return ex

                    def emit_av(stage, ex, first, last):
                        kind, a = stage
                        if kind == "pair":
                            for h_ in (0, 1):
                                j = a + h_
                                nc.tensor.matmul(
                                    otp[0 : D + 1, :],
                                    lhsT=vsb[:, j, :],
                                    rhs=ex[:, h_ * QB : (h_ + 1) * QB],
                                    start=False,
                                    stop=last and h_ == 1,
                                    skip_group_check=True,
                                )
                        else:
                            off = 0
                            for r_ in (a, a + 1):
                                j = nfull + r_
                                m = KB * r_
                                n = QB - m
                                nc.tensor.matmul(
                                    otp[0 : D + 1, m:QB],
                                    lhsT=vsb[:, j, :],
                                    rhs=ex[:, off : off + n],
                                    # r=0 spans the full [0:QB): it zeroes the
                                    # accumulation range for everything after
                                    start=first and r_ == a == 0,
                                    stop=last and r_ == a + 1 and nfull == 0,
                                    skip_group_check=True,
                                )
                                off += n

                    # software pipeline: 3 stages in flight keep the PE fed
                    # while the two exp engines work; the previous q-block's
                    # softmax drain is flushed mid-pipeline so the PE queue
                    # never waits on it
                    LAG = 3
                    nst = len(stages)
                    exs = {}
                    flushed = False
                    for t in range(nst + LAG):
                        if t < nst:
                            exs[t] = emit_s(stages[t])
                        # flush once an exp has run and the NEXT stage's exp
                        # goes to VectorE (odd parity): the drain copy then
                        # slots into the scalar engine's idle window instead
                        # of delaying an in-flight Exp
                        if not flushed and t >= 1 and (stage_ctr[0] % 2 == 1 or t >= 2):
                            flush_drain()
                            if qi == 1 and bh + 1 < BHPC:
                                nxt[0] = emit_inputs(bh + 1)
                            flushed = True
                        if t >= LAG:
                            s_ = t - LAG
                            emit_av(
                                stages[s_], exs.pop(s_),
                                first=(s_ == 0), last=(s_ == nst - 1),
                            )

                    def make_drain(bh=bh, qi=qi, otp=otp, ost=ost):
                        # the very last drain has nothing to hide behind: run
                        # it in halves so its DVE/DMA tail overlaps the other
                        # half's scalar/PE work
                        halves = (
                            [(0, 2), (2, 4)]
                            if bh == BHPC - 1 and qi == NQB - 1
                            else [(0, KB_PER_QB)]
                        )

                        def drain():
                            for c0, c1 in halves:
                                ncb = c1 - c0
                                otsb = otsb_pool.tile([D + 1, QB], f32, tag="otsb")
                                nc.vector.tensor_copy(
                                    otsb[:, 0 : ncb * 128],
                                    otp[0 : D + 1, c0 * 128 : c1 * 128],
                                )
                                # transpose scratch aliases the drained otp bank
                                for c_ in range(ncb):
                                    nc.tensor.transpose(
                                        otp[:, (c0 + c_) * (D + 1) : (c0 + c_ + 1) * (D + 1)],
                                        otsb[:, c_ * 128 : (c_ + 1) * 128],
                                        ident[0 : D + 1, 0 : D + 1],
                                    )
                                tp = otp[
                                    :, c0 * (D + 1) : c1 * (D + 1)
                                ].rearrange("p (c e) -> p c e", c=ncb)
                                rc = rc_pool.tile([128, ncb, 1], f32, tag="rc")
                                nc.vector.reciprocal(rc[:, :, 0], tp[:, :, D])
                                nc.vector.tensor_mul(
                                    ost[:, qi * KB_PER_QB + c0 : qi * KB_PER_QB + c1, :],
                                    tp[:, :, 0:D],
                                    rc[:].broadcast_to([128, ncb, D]),
                                )
                                # stream this q-block's output rows out
                                nc.sync.dma_start(
                                    out_d[bh]
                                    .rearrange("(n p) d -> p n d", p=128)[
                                        :, qi * KB_PER_QB + c0 : qi * KB_PER_QB + c1, :
                                    ],
                                    ost[:, qi * KB_PER_QB + c0 : qi * KB_PER_QB + c1, :],
                                )

                        return drain

                    pending_drain[0] = make_drain()
                cur, nxt[0] = nxt[0], None
            flush_drain()

    nc.compile()
    return nc


def _host_consts():
    kk = np.arange(128)[:, None]
    cc = np.arange(128)[None, :]
    triu = (kk > cc).astype(np.float16)  # 1 on masked (future) positions
    diagneg = (-320.0 * np.eye(128)).astype(np.float16)
    ident = np.eye(128, dtype=np.float32)
    return triu, diagneg, ident


def _shard_inputs(q, k, v, pe_q, pe_k):
    q = np.asarray(q, dtype=np.float32).reshape(B * H, L, D)
    k = np.asarray(k, dtype=np.float32).reshape(B * H, L, D)
    v = np.asarray(v, dtype=np.float32).reshape(B * H, L, D)
    pe_q = np.asarray(pe_q, dtype=np.float32).reshape(B * H, L, D)
    pe_k = np.asarray(pe_k, dtype=np.float32).reshape(B * H, L, D)
    # layout packing only (no math): concat dual-score operands, pre-transpose
    # to the d-major layout the PE contracts over, cast to the f16 the matmuls
    # run in, and bake the ones-column into V for the denominator trick
    qT = np.ascontiguousarray(
        np.concatenate([q, pe_q], axis=-1).transpose(0, 2, 1)
    ).astype(np.float16)
    kT = np.ascontiguousarray(
        np.concatenate([k, pe_k], axis=-1).transpose(0, 2, 1)
    ).astype(np.float16)
    vone = np.ones((B * H, 128, NKB, D + 1), dtype=np.float16)
    vone[:, :, :, 0:D] = v.reshape(B * H, NKB, 128, D).transpose(0, 2, 1, 3)
    triu, diagneg, ident = _host_consts()
    in_maps = []
    for c in range(NCORES):
        s = slice(c * BHPC, (c + 1) * BHPC)
        in_maps.append(
            {
                "qT": qT[s],
                "kT": kT[s],
                "v": vone[s],
                "triu": triu,
                "diagneg": diagneg,
                "ident": ident,
            }
        )
    return in_maps


def kernel(q, k, v, pe_q, pe_k, mask=None, **_ignored):
    """Full-input entry point: shards across 8 NeuronCores, returns full output.

    The mask input is the (fixed) causal mask of the problem; causality is
    implemented structurally in the device kernel, so it is not shipped.
    """
    _import_concourse()
    from concourse.bass_utils import run_bass_kernel_spmd

    if "nc" not in _CACHE:
        _CACHE["nc"] = _build_nc()
    nc = _CACHE["nc"]

    in_maps = _shard_inputs(q, k, v, pe_q, pe_k)
    res = run_bass_kernel_spmd(nc, in_maps, core_ids=list(range(NCORES)))
    out = np.empty((B * H, L, D), dtype=np.float32)
    for c in range(NCORES):
        out[c * BHPC : (c + 1) * BHPC] = res.results[c]["out"]
    return out.reshape(B, H, L, D)


# revision 30
# speedup vs baseline: 1.1404x; 1.1404x over previous
"""Dual-score causal attention on 8 Trainium2 NeuronCores.

Math (per batch*head):
    S = (q @ k.T + pe_q @ pe_k.T) * D**-0.5   == concat(q,pe_q) @ concat(k,pe_k).T * scale
    O = softmax(causal_mask(S)) @ v

Sharding: B*H = 32 pairs -> 4 per core (head/data parallel, no collectives).

Layout strategy (host-side shard step; pure data movement, no math):
  - Q' = [q|pe_q], K' = [k|pe_k] pre-transposed to d-major [128, L] f16 so the
    device streams them into SBUF at line rate (no on-device staging/transpose).
  - V laid out [128, NKB, 65] f16 with a ones-column baked in (row sums of
    exp(S) fall out of the A@V matmul chain -> softmax denominator).

Per-core kernel (engine-balanced, latency-hidden):
  - PE: S^T tiles [128 k x 512 q] fp16 -> PSUM; A^T@V accumulate [65, 512];
    O^T transposed back via identity matmul.
  - exp alternates between the scalar engine (table Exp) and VectorE
    (single-pass Schraudolph: ex_bits = i16(s*A + B) reinterpreted as f16)
    per stage, so both engines run concurrently at ~50% duty.  The softmax
    normalization cancels the approximation's bias (end-to-end ~7e-3 vs the
    2e-2 gate).
  - 3-deep PSUM score pipeline (stp bufs=3, LAG=3) hides the S->exp->AV
    cross-engine semaphore latency.  Diagonal (small) stages lead each
    q-block so the following full-size stages provide maximum latency cover.
  - Causal masking costs no elementwise work at all: one extra 128-col matmul
    per diagonal block accumulates -320 onto the upper triangle in PSUM
    (lhsT=diag(-320), rhs=upper-tri indicator).  exp then emits exact zeros:
    the scalar engine's Exp underflows, and the DVE path's f32->i16 convert
    saturates to -32768 whose f16 bitcast is -0.0 (verified on hardware).
  - A short burst of discarded matmuls during the input-DMA wait ramps the
    PE out of its cold p-state before the first real score matmul.
  - PSUM budget trick: the O^T transpose scratch aliases into the otp tile
    itself (already copied to SBUF by then): stp 3x2 + otp 2x1 = all 8 banks.
  - Drains are deferred one stage into the next q-block; outputs stream per
    q-block; next-bh input DMAs prefetch a full q-block ahead.
"""

import os
import sys

import numpy as np

B, H, L, D = 2, 16, 2048, 64
NCORES = 8
BHPC = (B * H) // NCORES  # bh pairs per core = 4
QB = 512  # query block (S^T free dim)
KB = 128  # key block (S^T partition dim)
NQB = L // QB  # 4
NKB = L // KB  # 16
KB_PER_QB = QB // KB  # 4
SCALE = float(D) ** -0.5
LOG2E = 1.4426950408889634
# Schraudolph-in-f16-bits: exp(s*SCALE) ~ bitcast_f16(int16(s*A + B))
SCHR_A = SCALE * LOG2E * 1024.0
SCHR_B = 15.0 * 1024.0

_CACHE = {}


def _import_concourse():
    try:
        import concourse  # noqa: F401
    except ImportError:
        for p in ("/opt/trn_rl_repo", "/root/.axon_site/_ro/trn_rl_repo"):
            if os.path.isdir(p) and p not in sys.path:
                sys.path.insert(0, p)


def _build_nc():
    """Build the single-core Bass program (same NEFF for all 8 cores)."""
    _import_concourse()
    from contextlib import ExitStack

    import concourse.tile as tile
    from concourse import bacc, mybir

    f32 = mybir.dt.float32
    f16 = mybir.dt.float16
    i16 = mybir.dt.int16

    nc = bacc.Bacc("TRN2", target_bir_lowering=False, debug=False)

    qT_d = nc.dram_tensor("qT", [BHPC, 128, L], f16, kind="ExternalInput").ap()
    kT_d = nc.dram_tensor("kT", [BHPC, 128, L], f16, kind="ExternalInput").ap()
    v_d = nc.dram_tensor("v", [BHPC, 128, NKB, D + 1], f16, kind="ExternalInput").ap()
    triu_d = nc.dram_tensor("triu", [128, 128], f16, kind="ExternalInput").ap()
    diagneg_d = nc.dram_tensor("diagneg", [128, 128], f16, kind="ExternalInput").ap()
    ident_d = nc.dram_tensor("ident", [128, 128], f32, kind="ExternalInput").ap()
    out_d = nc.dram_tensor("out", [BHPC, L, D], f32, kind="ExternalOutput").ap()

    Exp = mybir.ActivationFunctionType.Exp
    mult = mybir.AluOpType.mult
    add = mybir.AluOpType.add

    with tile.TileContext(nc) as tc:
        with ExitStack() as ctx:
            ep = ctx.enter_context

            const_pool = ep(tc.tile_pool(name="const", bufs=1))
            qT_pool = ep(tc.tile_pool(name="qT", bufs=2))
            kT_pool = ep(tc.tile_pool(name="kT", bufs=2))
            v_pool = ep(tc.tile_pool(name="v", bufs=2))
            ex_pool = ep(tc.tile_pool(name="ex", bufs=6))
            otsb_pool = ep(tc.tile_pool(name="otsb", bufs=2))
            ost_pool = ep(tc.tile_pool(name="ost", bufs=2))
            rc_pool = ep(tc.tile_pool(name="rc", bufs=4))
            stp_pool = ep(tc.tile_pool(name="stp", bufs=3, space="PSUM"))
            otp_pool = ep(tc.tile_pool(name="otp", bufs=2, space="PSUM"))

            triu = const_pool.tile([128, 128], f16)
            diagneg = const_pool.tile([128, 128], f16)
            ident = const_pool.tile([128, 128], f32)
            warm = const_pool.tile([128, 1], f32)
            nc.vector.memset(warm[:], 0.0)
            wpe = const_pool.tile([128, QB], f16)
            nc.vector.memset(wpe[:], 1.0)

            def emit_inputs(bh):
                qT = qT_pool.tile([128, L], f16)
                kT = kT_pool.tile([128, L], f16)
                vsb = v_pool.tile([128, NKB, D + 1], f16)
                if bh == 0:
                    # startup: what qi=0 stage-0 needs lands first, spread
                    # over two trigger queues so the transfers parallelize;
                    # the big qT chunk rides sync (earliest-clearing queue)
                    # scalar queue gets only the three tiny first-need
                    # transfers (so its exp table-load + first Exp aren't
                    # starved); everything else streams on sync
                    nc.sync.dma_start(qT[:, 0 : QB // 2], qT_d[bh, :, 0 : QB // 2])
                    nc.scalar.dma_start(kT[:, 0:KB], kT_d[bh, :, 0:KB])
                    nc.scalar.dma_start(triu[:], triu_d)
                    nc.scalar.dma_start(diagneg[:], diagneg_d)
                    nc.sync.dma_start(qT[:, QB // 2 : QB], qT_d[bh, :, QB // 2 : QB])
                    nc.scalar.dma_start(kT[:, KB:QB], kT_d[bh, :, KB:QB])
                    nc.scalar.dma_start(vsb[:], v_d[bh])
                    nc.scalar.dma_start(ident[:], ident_d)
                    nc.sync.dma_start(kT[:, QB:L], kT_d[bh, :, QB:L])
                    nc.sync.dma_start(qT[:, QB:L], qT_d[bh, :, QB:L])
                else:
                    nc.sync.dma_start(kT[:], kT_d[bh])
                    nc.sync.dma_start(qT[:], qT_d[bh])
                    nc.sync.dma_start(vsb[:], v_d[bh])
                return qT, kT, vsb

            pending_drain = [None]

            def flush_drain():
                if pending_drain[0] is not None:
                    pending_drain[0]()
                    pending_drain[0] = None

            stage_ctr = [0]  # global ACT/DVE alternation parity
            cur = emit_inputs(0)
            # preload the Exp activation table while the first DMAs land (the
            # implicit load otherwise delays the first real exp by ~1.4us);
            # emitted after the input triggers so it doesn't delay them
            nc.scalar.activation(warm[:], warm[:], Exp)
            # ramp the PE p-state during the input-DMA wait with discarded
            # self-contained matmuls so the first real score matmuls run at
            # full clock
            for _ in range(2):
                wstp = stp_pool.tile([128, 2 * QB], f32, tag="stp")
                for h_ in range(3):
                    nc.tensor.matmul(
                        wstp[:, (h_ % 2) * QB : (h_ % 2 + 1) * QB],
                        lhsT=wpe[:, 0:KB],
                        rhs=wpe[:],
                        start=True,
                        stop=True,
                        skip_group_check=True,
                    )
            nxt = [None]

            for bh in range(BHPC):
                qT, kT, vsb = cur
                ost = ost_pool.tile([128, NKB, D], f32)
                for qi in range(NQB):
                    # otp spans all 128 partitions: [0:65] holds the O^T
                    # accumulation; after its drain-copy the same bank is
                    # reused as transpose scratch (PSUM is only 8 banks)
                    otp = otp_pool.tile([128, QB], f32)
                    nfull = KB_PER_QB * qi  # fully-unmasked k-blocks

                    def emit_exp(ex, stp, ncols, is_dpair):
                        # -320 mask rows rely on the DVE f32->i16 convert
                        # SATURATING to -32768 (bitcast f16 -0.0)
                        on_dve = stage_ctr[0] % 2 == 1
                        stage_ctr[0] += 1
                        if on_dve:
                            nc.vector.tensor_scalar(
                                ex[:, 0:ncols].bitcast(i16),
                                stp[:, 0:ncols],
                                SCHR_A,
                                SCHR_B,
                                mult,
                                add,
                            )
                        else:
                            nc.scalar.activation(
                                ex[:, 0:ncols], stp[:, 0:ncols], Exp, scale=SCALE
                            )

                    # diagonal (small) stages lead: the following full-size
                    # stages then cover their exp/triangle latency, and the
                    # first dpair's r=0 matmul initializes the whole otp range
                    stages = [("dpair", r0) for r0 in range(0, KB_PER_QB, 2)]
                    stages += [("pair", j0) for j0 in range(0, nfull, 2)]

                    def emit_s(stage):
                        kind, a = stage
                        stp = stp_pool.tile([128, 2 * QB], f32, tag="stp")
                        ex = ex_pool.tile([128, 2 * QB], f16, tag="ex")
                        if kind == "pair":
                            for h_ in (0, 1):
                                j = a + h_
                                nc.tensor.matmul(
                                    stp[:, h_ * QB : (h_ + 1) * QB],
                                    lhsT=kT[:, j * KB : (j + 1) * KB],
                                    rhs=qT[:, qi * QB : (qi + 1) * QB],
                                    start=True,
                                    stop=True,
                                    skip_group_check=True,
                                )
                            emit_exp(ex, stp, 2 * QB, False)
                        else:
                            # two diagonal blocks r0, r0+1 packed into one
                            # exp: [0:na) for r0, [na:na+nb) for r0+1
                            off = 0
                            for r_ in (a, a + 1):
                                j = nfull + r_
                                m = KB * r_
                                n = QB - m
                                nc.tensor.matmul(
                                    stp[:, off : off + n],
                                    lhsT=kT[:, j * KB : (j + 1) * KB],
                                    rhs=qT[:, qi * QB + m : (qi + 1) * QB],
                                    start=True,
                                    stop=False,
                                    skip_group_check=True,
                                )
                                # causal mask: accumulate -320 onto the diag
                                # block's upper triangle (exp -> exact 0),
                                # replacing a VectorE/GpSimd multiply chain
                                nc.tensor.matmul(
                                    stp[:, off : off + KB],
                                    lhsT=diagneg[:],
                                    rhs=triu[:],
                                    start=False,
                                    stop=True,
                                    skip_group_check=True,
                                )
                                off += n
                            emit_exp(ex, stp, off, True)
                        return ex

                    def emit_av(stage, ex, first, last):
                        kind, a = stage
                        if kind == "pair":
                            for h_ in (0, 1):
                                j = a + h_
                                nc.tensor.matmul(
                                    otp[0 : D + 1, :],
                                    lhsT=vsb[:, j, :],
                                    rhs=ex[:, h_ * QB : (h_ + 1) * QB],
                                    start=False,
                                    stop=last and h_ == 1,
                                    skip_group_check=True,
                                )
                        else:
                            off = 0
                            for r_ in (a, a + 1):
                                j = nfull + r_
                                m = KB * r_
                                n = QB - m
                                nc.tensor.matmul(
                                    otp[0 : D + 1, m:QB],
                                    lhsT=vsb[:, j, :],
                                    rhs=ex[:, off : off + n],
                                    # r=0 spans the full [0:QB): it zeroes the
                                    # accumulation range for everything after
                                    start=first and r_ == a == 0,
                                    stop=last and r_ == a + 1 and nfull == 0,
                                    skip_group_check=True,
                                )
                                off += n

                    # software pipeline: 3 stages in flight keep the PE fed
                    # while the two exp engines work; the previous q-block's
                    # softmax drain is flushed mid-pipeline so the PE queue
                    # never waits on it
                    LAG = 3
                    nst = len(stages)
                    exs = {}
                    flushed = False
                    for t in range(nst + LAG):
                        if t < nst:
                            exs[t] = emit_s(stages[t])
                        # flush once an exp has run and the NEXT stage's exp
                        # goes to VectorE (odd parity): the drain copy then
                        # slots into the scalar engine's idle window instead
                        # of delaying an in-flight Exp
                        if not flushed and t >= 1 and (stage_ctr[0] % 2 == 1 or t >= 2):
                            flush_drain()
                            if qi == 1 and bh + 1 < BHPC:
                                nxt[0] = emit_inputs(bh + 1)
                            flushed = True
                        if t >= LAG:
                            s_ = t - LAG
                            emit_av(
                                stages[s_], exs.pop(s_),
                                first=(s_ == 0), last=(s_ == nst - 1),
                            )

                    def make_drain(bh=bh, qi=qi, otp=otp, ost=ost):
                        # the very last drain has nothing to hide behind: run
                        # it in halves so its DVE/DMA tail overlaps the other
                        # half's scalar/PE work
                        halves = (
                            [(c, c + 1) for c in range(KB_PER_QB)]
                            if bh == BHPC - 1 and qi == NQB - 1
                            else [(0, KB_PER_QB)]
                        )

                        def drain():
                            for c0, c1 in halves:
                                ncb = c1 - c0
                                otsb = otsb_pool.tile([D + 1, QB], f32, tag="otsb")
                                nc.vector.tensor_copy(
                                    otsb[:, 0 : ncb * 128],
                                    otp[0 : D + 1, c0 * 128 : c1 * 128],
                                )
                                # transpose scratch aliases the drained otp bank
                                for c_ in range(ncb):
                                    nc.tensor.transpose(
                                        otp[:, (c0 + c_) * (D + 1) : (c0 + c_ + 1) * (D + 1)],
                                        otsb[:, c_ * 128 : (c_ + 1) * 128],
                                        ident[0 : D + 1, 0 : D + 1],
                                    )
                                tp = otp[
                                    :, c0 * (D + 1) : c1 * (D + 1)
                                ].rearrange("p (c e) -> p c e", c=ncb)
                                rc = rc_pool.tile([128, ncb, 1], f32, tag="rc")
                                nc.vector.reciprocal(rc[:, :, 0], tp[:, :, D])
                                nc.vector.tensor_mul(
                                    ost[:, qi * KB_PER_QB + c0 : qi * KB_PER_QB + c1, :],
                                    tp[:, :, 0:D],
                                    rc[:].broadcast_to([128, ncb, D]),
                                )
                                # stream this q-block's output rows out
                                nc.sync.dma_start(
                                    out_d[bh]
                                    .rearrange("(n p) d -> p n d", p=128)[
                                        :, qi * KB_PER_QB + c0 : qi * KB_PER_QB + c1, :
                                    ],
                                    ost[:, qi * KB_PER_QB + c0 : qi * KB_PER_QB + c1, :],
                                )

                        return drain

                    pending_drain[0] = make_drain()
                cur, nxt[0] = nxt[0], None
            flush_drain()

    nc.compile()
    return nc


def _host_consts():
    kk = np.arange(128)[:, None]
    cc = np.arange(128)[None, :]
    triu = (kk > cc).astype(np.float16)  # 1 on masked (future) positions
    diagneg = (-320.0 * np.eye(128)).astype(np.float16)
    ident = np.eye(128, dtype=np.float32)
    return triu, diagneg, ident


def _shard_inputs(q, k, v, pe_q, pe_k):
    q = np.asarray(q, dtype=np.float32).reshape(B * H, L, D)
    k = np.asarray(k, dtype=np.float32).reshape(B * H, L, D)
    v = np.asarray(v, dtype=np.float32).reshape(B * H, L, D)
    pe_q = np.asarray(pe_q, dtype=np.float32).reshape(B * H, L, D)
    pe_k = np.asarray(pe_k, dtype=np.float32).reshape(B * H, L, D)
    # layout packing only (no math): concat dual-score operands, pre-transpose
    # to the d-major layout the PE contracts over, cast to the f16 the matmuls
    # run in, and bake the ones-column into V for the denominator trick
    qT = np.ascontiguousarray(
        np.concatenate([q, pe_q], axis=-1).transpose(0, 2, 1)
    ).astype(np.float16)
    kT = np.ascontiguousarray(
        np.concatenate([k, pe_k], axis=-1).transpose(0, 2, 1)
    ).astype(np.float16)
    vone = np.ones((B * H, 128, NKB, D + 1), dtype=np.float16)
    vone[:, :, :, 0:D] = v.reshape(B * H, NKB, 128, D).transpose(0, 2, 1, 3)
    triu, diagneg, ident = _host_consts()
    in_maps = []
    for c in range(NCORES):
        s = slice(c * BHPC, (c + 1) * BHPC)
        in_maps.append(
            {
                "qT": qT[s],
                "kT": kT[s],
                "v": vone[s],
                "triu": triu,
                "diagneg": diagneg,
                "ident": ident,
            }
        )
    return in_maps


def kernel(q, k, v, pe_q, pe_k, mask=None, **_ignored):
    """Full-input entry point: shards across 8 NeuronCores, returns full output.

    The mask input is the (fixed) causal mask of the problem; causality is
    implemented structurally in the device kernel, so it is not shipped.
    """
    _import_concourse()
    from concourse.bass_utils import run_bass_kernel_spmd

    if "nc" not in _CACHE:
        _CACHE["nc"] = _build_nc()
    nc = _CACHE["nc"]

    in_maps = _shard_inputs(q, k, v, pe_q, pe_k)
    res = run_bass_kernel_spmd(nc, in_maps, core_ids=list(range(NCORES)))
    out = np.empty((B * H, L, D), dtype=np.float32)
    for c in range(NCORES):
        out[c * BHPC : (c + 1) * BHPC] = res.results[c]["out"]
    return out.reshape(B, H, L, D)


# revision 31
# speedup vs baseline: 1.2004x; 1.0526x over previous
"""Dual-score causal attention on 8 Trainium2 NeuronCores.

Math (per batch*head):
    S = (q @ k.T + pe_q @ pe_k.T) * D**-0.5   == concat(q,pe_q) @ concat(k,pe_k).T * scale
    O = softmax(causal_mask(S)) @ v

Sharding: B*H = 32 pairs -> 4 per core (head/data parallel, no collectives).

Layout strategy (host-side shard step; pure data movement, no math):
  - Q' = [q|pe_q], K' = [k|pe_k] pre-transposed to d-major [128, L] f16 so the
    device streams them into SBUF at line rate (no on-device staging/transpose).
  - V laid out [128, NKB, 65] f16 with a ones-column baked in (row sums of
    exp(S) fall out of the A@V matmul chain -> softmax denominator).

Per-core kernel (engine-balanced, latency-hidden):
  - PE: S^T tiles [128 k x 512 q] fp16 -> PSUM; A^T@V accumulate [65, 512];
    O^T transposed back via identity matmul.
  - exp alternates between the scalar engine (table Exp) and VectorE
    (single-pass Schraudolph: ex_bits = i16(s*A + B) reinterpreted as f16)
    per stage, so both engines run concurrently at ~50% duty.  The softmax
    normalization cancels the approximation's bias (end-to-end ~7e-3 vs the
    2e-2 gate).
  - 3-deep PSUM score pipeline (stp bufs=3, LAG=3) hides the S->exp->AV
    cross-engine semaphore latency.  Diagonal (small) stages lead each
    q-block so the following full-size stages provide maximum latency cover.
  - Causal masking costs no elementwise work at all: one extra 128-col matmul
    per diagonal block accumulates -320 onto the upper triangle in PSUM
    (lhsT=diag(-320), rhs=upper-tri indicator).  exp then emits exact zeros:
    the scalar engine's Exp underflows, and the DVE path's f32->i16 convert
    saturates to -32768 whose f16 bitcast is -0.0 (verified on hardware).
  - A short burst of discarded matmuls during the input-DMA wait ramps the
    PE out of its cold p-state before the first real score matmul.
  - PSUM budget trick: the O^T transpose scratch aliases into the otp tile
    itself (already copied to SBUF by then): stp 3x2 + otp 2x1 = all 8 banks.
  - Drains are deferred one stage into the next q-block; outputs stream per
    q-block; next-bh input DMAs prefetch a full q-block ahead.
"""

import os
import sys

import numpy as np

B, H, L, D = 2, 16, 2048, 64
NCORES = 8
BHPC = (B * H) // NCORES  # bh pairs per core = 4
QB = 512  # query block (S^T free dim)
KB = 128  # key block (S^T partition dim)
NQB = L // QB  # 4
NKB = L // KB  # 16
KB_PER_QB = QB // KB  # 4
SCALE = float(D) ** -0.5
LOG2E = 1.4426950408889634
# Schraudolph-in-f16-bits: exp(s*SCALE) ~ bitcast_f16(int16(s*A + B))
SCHR_A = SCALE * LOG2E * 1024.0
SCHR_B = 15.0 * 1024.0

_CACHE = {}


def _import_concourse():
    try:
        import concourse  # noqa: F401
    except ImportError:
        for p in ("/opt/trn_rl_repo", "/root/.axon_site/_ro/trn_rl_repo"):
            if os.path.isdir(p) and p not in sys.path:
                sys.path.insert(0, p)


def _build_nc():
    """Build the single-core Bass program (same NEFF for all 8 cores)."""
    _import_concourse()
    from contextlib import ExitStack

    import concourse.tile as tile
    from concourse import bacc, mybir

    f32 = mybir.dt.float32
    f16 = mybir.dt.float16
    i16 = mybir.dt.int16

    nc = bacc.Bacc("TRN2", target_bir_lowering=False, debug=False)

    qT_d = nc.dram_tensor("qT", [BHPC, 128, L], f16, kind="ExternalInput").ap()
    kT_d = nc.dram_tensor("kT", [BHPC, 128, L], f16, kind="ExternalInput").ap()
    v_d = nc.dram_tensor("v", [BHPC, 128, NKB, D + 1], f16, kind="ExternalInput").ap()
    triu_d = nc.dram_tensor("triu", [128, 128], f16, kind="ExternalInput").ap()
    diagneg_d = nc.dram_tensor("diagneg", [128, 128], f16, kind="ExternalInput").ap()
    ident_d = nc.dram_tensor("ident", [128, 128], f32, kind="ExternalInput").ap()
    out_d = nc.dram_tensor("out", [BHPC, L, D], f32, kind="ExternalOutput").ap()

    Exp = mybir.ActivationFunctionType.Exp
    mult = mybir.AluOpType.mult
    add = mybir.AluOpType.add

    with tile.TileContext(nc) as tc:
        with ExitStack() as ctx:
            ep = ctx.enter_context

            const_pool = ep(tc.tile_pool(name="const", bufs=1))
            qT_pool = ep(tc.tile_pool(name="qT", bufs=2))
            kT_pool = ep(tc.tile_pool(name="kT", bufs=2))
            v_pool = ep(tc.tile_pool(name="v", bufs=2))
            ex_pool = ep(tc.tile_pool(name="ex", bufs=6))
            otsb_pool = ep(tc.tile_pool(name="otsb", bufs=2))
            ost_pool = ep(tc.tile_pool(name="ost", bufs=2))
            rc_pool = ep(tc.tile_pool(name="rc", bufs=4))
            stp_pool = ep(tc.tile_pool(name="stp", bufs=3, space="PSUM"))
            otp_pool = ep(tc.tile_pool(name="otp", bufs=2, space="PSUM"))

            triu = const_pool.tile([128, 128], f16)
            diagneg = const_pool.tile([128, 128], f16)
            ident = const_pool.tile([128, 128], f32)
            warm = const_pool.tile([128, 1], f32)
            nc.vector.memset(warm[:], 0.0)
            wpe = const_pool.tile([128, QB], f16)
            nc.vector.memset(wpe[:], 1.0)

            def emit_inputs(bh):
                qT = qT_pool.tile([128, L], f16)
                kT = kT_pool.tile([128, L], f16)
                vsb = v_pool.tile([128, NKB, D + 1], f16)
                if bh == 0:
                    # startup: what qi=0 stage-0 needs lands first, spread
                    # over two trigger queues so the transfers parallelize;
                    # the big qT chunk rides sync (earliest-clearing queue)
                    # scalar queue gets only the three tiny first-need
                    # transfers (so its exp table-load + first Exp aren't
                    # starved); everything else streams on sync
                    nc.sync.dma_start(qT[:, 0 : QB // 2], qT_d[bh, :, 0 : QB // 2])
                    nc.scalar.dma_start(kT[:, 0:KB], kT_d[bh, :, 0:KB])
                    nc.scalar.dma_start(triu[:], triu_d)
                    nc.scalar.dma_start(diagneg[:], diagneg_d)
                    nc.sync.dma_start(qT[:, QB // 2 : QB], qT_d[bh, :, QB // 2 : QB])
                    nc.scalar.dma_start(kT[:, KB:QB], kT_d[bh, :, KB:QB])
                    nc.scalar.dma_start(vsb[:], v_d[bh])
                    nc.scalar.dma_start(ident[:], ident_d)
                    nc.sync.dma_start(kT[:, QB:L], kT_d[bh, :, QB:L])
                    nc.sync.dma_start(qT[:, QB:L], qT_d[bh, :, QB:L])
                else:
                    nc.sync.dma_start(kT[:], kT_d[bh])
                    nc.sync.dma_start(qT[:], qT_d[bh])
                    nc.sync.dma_start(vsb[:], v_d[bh])
                return qT, kT, vsb

            pending_drain = [None]

            def flush_drain():
                if pending_drain[0] is not None:
                    pending_drain[0]()
                    pending_drain[0] = None

            stage_ctr = [0]  # global ACT/DVE alternation parity
            cur = emit_inputs(0)
            # preload the Exp activation table while the first DMAs land (the
            # implicit load otherwise delays the first real exp by ~1.4us);
            # emitted after the input triggers so it doesn't delay them
            nc.scalar.activation(warm[:], warm[:], Exp)
            # ramp the PE p-state during the input-DMA wait with discarded
            # self-contained matmuls so the first real score matmuls run at
            # full clock
            for _ in range(2):
                wstp = stp_pool.tile([128, 2 * QB], f32, tag="stp")
                for h_ in range(4):
                    nc.tensor.matmul(
                        wstp[:, (h_ % 2) * QB : (h_ % 2 + 1) * QB],
                        lhsT=wpe[:, 0:KB],
                        rhs=wpe[:],
                        start=True,
                        stop=True,
                        skip_group_check=True,
                    )
            nxt = [None]

            for bh in range(BHPC):
                qT, kT, vsb = cur
                ost = ost_pool.tile([128, NKB, D], f32)
                for qi in range(NQB):
                    # otp spans all 128 partitions: [0:65] holds the O^T
                    # accumulation; after its drain-copy the same bank is
                    # reused as transpose scratch (PSUM is only 8 banks)
                    otp = otp_pool.tile([128, QB], f32)
                    nfull = KB_PER_QB * qi  # fully-unmasked k-blocks

                    def emit_exp(ex, stp, ncols, is_dpair):
                        # -320 mask rows rely on the DVE f32->i16 convert
                        # SATURATING to -32768 (bitcast f16 -0.0)
                        on_dve = stage_ctr[0] % 2 == 1
                        stage_ctr[0] += 1
                        if on_dve:
                            nc.vector.tensor_scalar(
                                ex[:, 0:ncols].bitcast(i16),
                                stp[:, 0:ncols],
                                SCHR_A,
                                SCHR_B,
                                mult,
                                add,
                            )
                        else:
                            nc.scalar.activation(
                                ex[:, 0:ncols], stp[:, 0:ncols], Exp, scale=SCALE
                            )

                    # diagonal (small) stages lead: the following full-size
                    # stages then cover their exp/triangle latency, and the
                    # first dpair's r=0 matmul initializes the whole otp range
                    stages = [("dpair", r0) for r0 in range(0, KB_PER_QB, 2)]
                    stages += [("pair", j0) for j0 in range(0, nfull, 2)]

                    def emit_s(stage):
                        kind, a = stage
                        stp = stp_pool.tile([128, 2 * QB], f32, tag="stp")
                        ex = ex_pool.tile([128, 2 * QB], f16, tag="ex")
                        if kind == "pair":
                            for h_ in (0, 1):
                                j = a + h_
                                nc.tensor.matmul(
                                    stp[:, h_ * QB : (h_ + 1) * QB],
                                    lhsT=kT[:, j * KB : (j + 1) * KB],
                                    rhs=qT[:, qi * QB : (qi + 1) * QB],
                                    start=True,
                                    stop=True,
                                    skip_group_check=True,
                                )
                            emit_exp(ex, stp, 2 * QB, False)
                        else:
                            # two diagonal blocks r0, r0+1 packed into one
                            # exp: [0:na) for r0, [na:na+nb) for r0+1
                            off = 0
                            for r_ in (a, a + 1):
                                j = nfull + r_
                                m = KB * r_
                                n = QB - m
                                nc.tensor.matmul(
                                    stp[:, off : off + n],
                                    lhsT=kT[:, j * KB : (j + 1) * KB],
                                    rhs=qT[:, qi * QB + m : (qi + 1) * QB],
                                    start=True,
                                    stop=False,
                                    skip_group_check=True,
                                )
                                # causal mask: accumulate -320 onto the diag
                                # block's upper triangle (exp -> exact 0),
                                # replacing a VectorE/GpSimd multiply chain
                                nc.tensor.matmul(
                                    stp[:, off : off + KB],
                                    lhsT=diagneg[:],
                                    rhs=triu[:],
                                    start=False,
                                    stop=True,
                                    skip_group_check=True,
                                )
                                off += n
                            emit_exp(ex, stp, off, True)
                        return ex

                    def emit_av(stage, ex, first, last):
                        kind, a = stage
                        if kind == "pair":
                            for h_ in (0, 1):
                                j = a + h_
                                nc.tensor.matmul(
                                    otp[0 : D + 1, :],
                                    lhsT=vsb[:, j, :],
                                    rhs=ex[:, h_ * QB : (h_ + 1) * QB],
                                    start=False,
                                    stop=last and h_ == 1,
                                    skip_group_check=True,
                                )
                        else:
                            off = 0
                            for r_ in (a, a + 1):
                                j = nfull + r_
                                m = KB * r_
                                n = QB - m
                                nc.tensor.matmul(
                                    otp[0 : D + 1, m:QB],
                                    lhsT=vsb[:, j, :],
                                    rhs=ex[:, off : off + n],
                                    # r=0 spans the full [0:QB): it zeroes the
                                    # accumulation range for everything after
                                    start=first and r_ == a == 0,
                                    stop=last and r_ == a + 1 and nfull == 0,
                                    skip_group_check=True,
                                )
                                off += n

                    # software pipeline: 3 stages in flight keep the PE fed
                    # while the two exp engines work; the previous q-block's
                    # softmax drain is flushed mid-pipeline so the PE queue
                    # never waits on it
                    LAG = 3
                    nst = len(stages)
                    exs = {}
                    flushed = False
                    for t in range(nst + LAG):
                        if t < nst:
                            exs[t] = emit_s(stages[t])
                        # flush once an exp has run and the NEXT stage's exp
                        # goes to VectorE (odd parity): the drain copy then
                        # slots into the scalar engine's idle window instead
                        # of delaying an in-flight Exp
                        if not flushed and t >= 1 and (stage_ctr[0] % 2 == 1 or t >= 2):
                            flush_drain()
                            if qi == 1 and bh + 1 < BHPC:
                                nxt[0] = emit_inputs(bh + 1)
                            flushed = True
                        if t >= LAG:
                            s_ = t - LAG
                            emit_av(
                                stages[s_], exs.pop(s_),
                                first=(s_ == 0), last=(s_ == nst - 1),
                            )

                    def make_drain(bh=bh, qi=qi, otp=otp, ost=ost):
                        # the very last drain has nothing to hide behind: run
                        # it in halves so its DVE/DMA tail overlaps the other
                        # half's scalar/PE work
                        halves = (
                            [(0, 2), (2, 4)]
                            if bh == BHPC - 1 and qi == NQB - 1
                            else [(0, KB_PER_QB)]
                        )

                        def drain():
                            for c0, c1 in halves:
                                ncb = c1 - c0
                                otsb = otsb_pool.tile([D + 1, QB], f32, tag="otsb")
                                nc.vector.tensor_copy(
                                    otsb[:, 0 : ncb * 128],
                                    otp[0 : D + 1, c0 * 128 : c1 * 128],
                                )
                                # transpose scratch aliases the drained otp bank
                                for c_ in range(ncb):
                                    nc.tensor.transpose(
                                        otp[:, (c0 + c_) * (D + 1) : (c0 + c_ + 1) * (D + 1)],
                                        otsb[:, c_ * 128 : (c_ + 1) * 128],
                                        ident[0 : D + 1, 0 : D + 1],
                                    )
                                tp = otp[
                                    :, c0 * (D + 1) : c1 * (D + 1)
                                ].rearrange("p (c e) -> p c e", c=ncb)
                                rc = rc_pool.tile([128, ncb, 1], f32, tag="rc")
                                nc.vector.reciprocal(rc[:, :, 0], tp[:, :, D])
                                nc.vector.tensor_mul(
                                    ost[:, qi * KB_PER_QB + c0 : qi * KB_PER_QB + c1, :],
                                    tp[:, :, 0:D],
                                    rc[:].broadcast_to([128, ncb, D]),
                                )
                                # stream this q-block's output rows out
                                nc.sync.dma_start(
                                    out_d[bh]
                                    .rearrange("(n p) d -> p n d", p=128)[
                                        :, qi * KB_PER_QB + c0 : qi * KB_PER_QB + c1, :
                                    ],
                                    ost[:, qi * KB_PER_QB + c0 : qi * KB_PER_QB + c1, :],
                                )

                        return drain

                    pending_drain[0] = make_drain()
                cur, nxt[0] = nxt[0], None
            flush_drain()

    nc.compile()
    return nc


def _host_consts():
    kk = np.arange(128)[:, None]
    cc = np.arange(128)[None, :]
    triu = (kk > cc).astype(np.float16)  # 1 on masked (future) positions
    diagneg = (-320.0 * np.eye(128)).astype(np.float16)
    ident = np.eye(128, dtype=np.float32)
    return triu, diagneg, ident


def _shard_inputs(q, k, v, pe_q, pe_k):
    q = np.asarray(q, dtype=np.float32).reshape(B * H, L, D)
    k = np.asarray(k, dtype=np.float32).reshape(B * H, L, D)
    v = np.asarray(v, dtype=np.float32).reshape(B * H, L, D)
    pe_q = np.asarray(pe_q, dtype=np.float32).reshape(B * H, L, D)
    pe_k = np.asarray(pe_k, dtype=np.float32).reshape(B * H, L, D)
    # layout packing only (no math): concat dual-score operands, pre-transpose
    # to the d-major layout the PE contracts over, cast to the f16 the matmuls
    # run in, and bake the ones-column into V for the denominator trick
    qT = np.ascontiguousarray(
        np.concatenate([q, pe_q], axis=-1).transpose(0, 2, 1)
    ).astype(np.float16)
    kT = np.ascontiguousarray(
        np.concatenate([k, pe_k], axis=-1).transpose(0, 2, 1)
    ).astype(np.float16)
    vone = np.ones((B * H, 128, NKB, D + 1), dtype=np.float16)
    vone[:, :, :, 0:D] = v.reshape(B * H, NKB, 128, D).transpose(0, 2, 1, 3)
    triu, diagneg, ident = _host_consts()
    in_maps = []
    for c in range(NCORES):
        s = slice(c * BHPC, (c + 1) * BHPC)
        in_maps.append(
            {
                "qT": qT[s],
                "kT": kT[s],
                "v": vone[s],
                "triu": triu,
                "diagneg": diagneg,
                "ident": ident,
            }
        )
    return in_maps


def kernel(q, k, v, pe_q, pe_k, mask=None, **_ignored):
    """Full-input entry point: shards across 8 NeuronCores, returns full output.

    The mask input is the (fixed) causal mask of the problem; causality is
    implemented structurally in the device kernel, so it is not shipped.
    """
    _import_concourse()
    from concourse.bass_utils import run_bass_kernel_spmd

    if "nc" not in _CACHE:
        _CACHE["nc"] = _build_nc()
    nc = _CACHE["nc"]

    in_maps = _shard_inputs(q, k, v, pe_q, pe_k)
    res = run_bass_kernel_spmd(nc, in_maps, core_ids=list(range(NCORES)))
    out = np.empty((B * H, L, D), dtype=np.float32)
    for c in range(NCORES):
        out[c * BHPC : (c + 1) * BHPC] = res.results[c]["out"]
    return out.reshape(B, H, L, D)
